# revision 9
# baseline (speedup 1.0000x reference)
import sys

if "/opt/trn_rl_repo" not in sys.path:
    sys.path.insert(0, "/opt/trn_rl_repo")

import numpy as np
from contextlib import ExitStack

from concourse import bass, bacc, tile

mybir = bass.mybir
FP = mybir.dt.float32
U32 = mybir.dt.uint32
AF = mybir.ActivationFunctionType
ALU = mybir.AluOpType

B, N, S = 4, 16384, 2048
D1, D2 = 128, 256
H1, H2 = 256, 128
NH = N // 2      # queries per core (batch b = c//2, half h = c%2)
NT = NH // 128   # 64 tiles of 128 queries
EPS = 1e-5
NCORES = 8


def _build():
    nc = bacc.Bacc("TRN2", target_bir_lowering=False)

    x1a_d = nc.dram_tensor("x1aug", (5, NH), FP, kind="ExternalInput")
    x2a_d = nc.dram_tensor("x2aug", (5, S), FP, kind="ExternalInput")
    p1_d = nc.dram_tensor("p1", (D1, NH), FP, kind="ExternalInput")
    p2t_d = nc.dram_tensor("p2t", (S, D2), FP, kind="ExternalInput")
    r1_d = nc.dram_tensor("rhs1", (385, H1), FP, kind="ExternalInput")
    r2_d = nc.dram_tensor("rhs2", (257, H2), FP, kind="ExternalInput")
    g1b_d = nc.dram_tensor("g1b", (128, H1), FP, kind="ExternalInput")
    be1b_d = nc.dram_tensor("be1b", (128, H1), FP, kind="ExternalInput")
    gb2_d = nc.dram_tensor("gb2", (H2, 2), FP, kind="ExternalInput")
    out_d = nc.dram_tensor("out", (H2, NH), FP, kind="ExternalOutput")

    from concourse.masks import make_identity

    with ExitStack() as ctx:
        tc = ctx.enter_context(tile.TileContext(nc))
        const = ctx.enter_context(tc.tile_pool(name="const", bufs=1))
        work = ctx.enter_context(tc.tile_pool(name="work", bufs=2))
        fpool = ctx.enter_context(tc.tile_pool(name="fpool", bufs=2))
        fps = ctx.enter_context(tc.tile_pool(name="fps", bufs=1, space="PSUM"))
        psmall = ctx.enter_context(tc.tile_pool(name="psmall", bufs=4, space="PSUM"))

        # ---------------- constants ----------------
        x1a = const.tile((5, NH), FP, tag="x1a")
        nc.sync.dma_start(x1a[:], x1a_d[:, :])
        x2a = const.tile((5, S), FP, tag="x2a")
        nc.sync.dma_start(x2a[:], x2a_d[:, :])
        r1a = const.tile((128, H1), FP, tag="r1a")
        nc.scalar.dma_start(r1a[:], r1_d[0:128, :])
        r1b = const.tile((128, H1), FP, tag="r1b")
        nc.scalar.dma_start(r1b[:], r1_d[128:256, :])
        r1c = const.tile((128, H1), FP, tag="r1c")
        nc.scalar.dma_start(r1c[:], r1_d[256:384, :])
        r1d = const.tile((1, H1), FP, tag="r1d")
        nc.scalar.dma_start(r1d[:], r1_d[384:385, :])
        r2a = const.tile((128, H2), FP, tag="r2a")
        nc.scalar.dma_start(r2a[:], r2_d[0:128, :])
        r2b = const.tile((128, H2), FP, tag="r2b")
        nc.scalar.dma_start(r2b[:], r2_d[128:256, :])
        r2c = const.tile((1, H2), FP, tag="r2c")
        nc.scalar.dma_start(r2c[:], r2_d[256:257, :])
        g1b = const.tile((128, H1), FP, tag="g1b")
        nc.sync.dma_start(g1b[:], g1b_d[:, :])
        be1b = const.tile((128, H1), FP, tag="be1b")
        nc.sync.dma_start(be1b[:], be1b_d[:, :])
        gb2 = const.tile((H2, 2), FP, tag="gb2")
        nc.scalar.dma_start(gb2[:], gb2_d[:, :])
        ident = const.tile((128, 128), FP, tag="ident")
        make_identity(nc, ident[:])
        ones1 = const.tile((1, 128), FP, tag="ones1")
        nc.gpsimd.memset(ones1[:], 1.0)
        epsb = const.tile((128, 1), FP, tag="epsb")
        nc.gpsimd.memset(epsb[:], EPS)

        state = [None] * NT

        def front(i):
            q0 = i * 128
            # f = -dist^2 = 2*x1.x2 - |x2|^2 - |x1|^2 via K=5 matmul
            f_ps = fps.tile((128, S), FP, tag="f_ps")
            for j in range(4):
                nc.tensor.matmul(
                    f_ps[:, j * 512:(j + 1) * 512],
                    x1a[:, q0:q0 + 128],
                    x2a[:, j * 512:(j + 1) * 512],
                    start=True, stop=True,
                )
            f_sb = fpool.tile((128, S), FP, tag="f_sb")
            nc.scalar.copy(f_sb[:], f_ps[:])
            # top-3 (largest f == smallest dist)
            v8 = work.tile((128, 8), FP, tag="v8")
            nc.vector.max(v8[:], f_sb[:])
            i8 = work.tile((128, 8), U32, tag="i8")
            nc.vector.max_index(i8[:], v8[:], f_sb[:])
            # w = (1/(d+1e-8)) normalized;  d = -v8
            dd = work.tile((128, 3), FP, tag="dd")
            nc.gpsimd.tensor_scalar(dd[:], v8[:, 0:3], -1.0, 1e-8, op0=ALU.mult, op1=ALU.add)
            rec3 = work.tile((128, 3), FP, tag="rec3")
            nc.vector.reciprocal(rec3[:], dd[:])
            rsum = work.tile((128, 1), FP, tag="rsum")
            nc.vector.tensor_reduce(rsum[:], rec3[:], axis=mybir.AxisListType.X, op=ALU.add)
            rn = work.tile((128, 1), FP, tag="rn")
            nc.vector.reciprocal(rn[:], rsum[:])
            w3 = work.tile((128, 3), FP, tag="w3")
            nc.gpsimd.tensor_scalar_mul(w3[:], rec3[:], rn[:])
            # gather 3 neighbor feature rows [128, 3*256] from p2t
            g_sb = work.tile((128, 3 * D2), FP, tag="g_sb")
            for k in range(3):
                nc.gpsimd.indirect_dma_start(
                    out=g_sb[:, k * D2:(k + 1) * D2],
                    out_offset=None,
                    in_=p2t_d[:, :],
                    in_offset=bass.IndirectOffsetOnAxis(ap=i8[:, k:k + 1], axis=0),
                )
            # prefetch points1 tile
            p1t = work.tile((128, 128), FP, tag="p1t")
            nc.sync.dma_start(p1t[:], p1_d[:, q0:q0 + 128])
            state[i] = (q0, w3, g_sb, p1t)

        def layernorm(h_ps, C):
            # returns xn = (h - mu) / sqrt(var + eps)   [128, C] SBUF
            inv = 1.0 / C
            h_sb = work.tile((128, C), FP, tag=f"h_sb{C}")
            s1 = work.tile((128, 1), FP, tag=f"s1_{C}")
            nc.scalar.activation(h_sb[:], h_ps[:], AF.Copy, bias=0.0, scale=1.0, accum_out=s1[:])
            sq = work.tile((128, C), FP, tag=f"sq{C}")
            s2 = work.tile((128, 1), FP, tag=f"s2_{C}")
            nc.scalar.activation(sq[:], h_ps[:], AF.Square, accum_out=s2[:])
            negmu = work.tile((128, 1), FP, tag=f"negmu{C}")
            nc.gpsimd.tensor_scalar_mul(negmu[:], s1[:], -inv)
            a1 = work.tile((128, 1), FP, tag=f"a1_{C}")
            nc.scalar.activation(a1[:], s1[:], AF.Square, scale=float(np.sqrt(inv)))
            vart = work.tile((128, 1), FP, tag=f"vart{C}")
            nc.gpsimd.tensor_sub(vart[:], s2[:], a1[:])
            sd = work.tile((128, 1), FP, tag=f"sd{C}")
            nc.scalar.activation(sd[:], vart[:], AF.Sqrt, bias=epsb[:], scale=inv)
            rs = work.tile((128, 1), FP, tag=f"rs{C}")
            nc.vector.reciprocal(rs[:], sd[:])
            hh = work.tile((128, C), FP, tag=f"hh{C}")
            nc.gpsimd.tensor_scalar_add(hh[:], h_sb[:], negmu[:])
            xn = work.tile((128, C), FP, tag=f"xn{C}")
            nc.gpsimd.tensor_scalar_mul(xn[:], hh[:], rs[:])
            return xn

        def back(i):
            q0, w3, g_sb, p1t = state[i]
            state[i] = None
            # weighted neighbor features
            t_sb = work.tile((128, 3 * D2), FP, tag="t_sb")
            for k in range(3):
                nc.scalar.mul(t_sb[:, k * D2:(k + 1) * D2], g_sb[:, k * D2:(k + 1) * D2], w3[:, k:k + 1])
            # interp^T via accumulating PE transposes: [256 d2 x 128 q] in two chunks
            it0 = psmall.tile((128, 128), FP, tag="ps")
            it1 = psmall.tile((128, 128), FP, tag="ps")
            for k in range(3):
                nc.tensor.matmul(it0[:], t_sb[:, k * D2:k * D2 + 128], ident[:],
                                 is_transpose=True, start=(k == 0), stop=(k == 2))
            for k in range(3):
                nc.tensor.matmul(it1[:], t_sb[:, k * D2 + 128:(k + 1) * D2], ident[:],
                                 is_transpose=True, start=(k == 0), stop=(k == 2))
            xt0 = work.tile((128, 128), FP, tag="xt0")
            nc.scalar.copy(xt0[:], it0[:])
            xt1 = work.tile((128, 128), FP, tag="xt1")
            nc.scalar.copy(xt1[:], it1[:])
            # MLP1: h1[q, o] = sum_c newpts[c, q] * W1[o, c] + b1[o]
            h1_ps = psmall.tile((128, H1), FP, tag="ps")
            nc.tensor.matmul(h1_ps[:], p1t[:], r1a[:], start=True, stop=False)
            nc.tensor.matmul(h1_ps[:], xt0[:], r1b[:], start=False, stop=False)
            nc.tensor.matmul(h1_ps[:], xt1[:], r1c[:], start=False, stop=False)
            nc.tensor.matmul(h1_ps[:], ones1[:], r1d[:], start=False, stop=True)
            xn1 = layernorm(h1_ps, H1)
            # affine (g1, be1) + relu on pool
            y1 = work.tile((128, H1), FP, tag="y1")
            nc.gpsimd.tensor_mul(y1[:], xn1[:], g1b[:])
            y2 = work.tile((128, H1), FP, tag="y2")
            nc.gpsimd.tensor_add(y2[:], y1[:], be1b[:])
            hn1 = work.tile((128, H1), FP, tag="hn1")
            nc.gpsimd.tensor_scalar_max(hn1[:], y2[:], 0.0)
            # transpose hn1 -> lhsT chunks for MLP2
            tp0 = psmall.tile((128, 128), FP, tag="ps")
            nc.tensor.matmul(tp0[:], hn1[:, 0:128], ident[:], is_transpose=True, start=True, stop=True)
            l2a = work.tile((128, 128), FP, tag="l2a")
            nc.scalar.copy(l2a[:], tp0[:])
            tp1 = psmall.tile((128, 128), FP, tag="ps")
            nc.tensor.matmul(tp1[:], hn1[:, 128:256], ident[:], is_transpose=True, start=True, stop=True)
            l2b = work.tile((128, 128), FP, tag="l2b")
            nc.scalar.copy(l2b[:], tp1[:])
            # MLP2
            h2_ps = psmall.tile((128, H2), FP, tag="ps")
            nc.tensor.matmul(h2_ps[:], l2a[:], r2a[:], start=True, stop=False)
            nc.tensor.matmul(h2_ps[:], l2b[:], r2b[:], start=False, stop=False)
            nc.tensor.matmul(h2_ps[:], ones1[:], r2c[:], start=False, stop=True)
            xn2 = layernorm(h2_ps, H2)
            # transpose to output layout, then g2/be2 affine + relu
            ot = psmall.tile((128, 128), FP, tag="ps")
            nc.tensor.matmul(ot[:], xn2[:], ident[:], is_transpose=True, start=True, stop=True)
            out_sb = work.tile((128, 128), FP, tag="out_sb")
            nc.scalar.activation(out_sb[:], ot[:], AF.Relu, bias=gb2[:, 1:2], scale=gb2[:, 0:1])
            nc.scalar.dma_start(out_d[:, q0:q0 + 128], out_sb[:])

        for i in range(NT):
            front(i)
            if i > 0:
                back(i - 1)
        back(NT - 1)

    nc.compile()
    return nc


_NC_CACHE = None


def _get_nc():
    global _NC_CACHE
    if _NC_CACHE is None:
        _NC_CACHE = _build()
    return _NC_CACHE


def _make_in_maps(inputs):
    xyz1 = np.asarray(inputs["xyz1"], np.float32)
    xyz2 = np.asarray(inputs["xyz2"], np.float32)
    points1 = np.asarray(inputs["points1"], np.float32)
    points2 = np.asarray(inputs["points2"], np.float32)
    W1 = np.asarray(inputs["W1"], np.float32)
    b1 = np.asarray(inputs["b1"], np.float32)
    g1 = np.asarray(inputs["g1"], np.float32)
    be1 = np.asarray(inputs["be1"], np.float32)
    W2 = np.asarray(inputs["W2"], np.float32)
    b2 = np.asarray(inputs["b2"], np.float32)
    g2 = np.asarray(inputs["g2"], np.float32)
    be2 = np.asarray(inputs["be2"], np.float32)

    rhs1 = np.ascontiguousarray(np.concatenate([W1.T, b1[None, :]], 0), dtype=np.float32)
    rhs2 = np.ascontiguousarray(np.concatenate([W2.T, b2[None, :]], 0), dtype=np.float32)
    g1bc = np.ascontiguousarray(np.broadcast_to(g1[None, :], (128, H1)), dtype=np.float32)
    be1bc = np.ascontiguousarray(np.broadcast_to(be1[None, :], (128, H1)), dtype=np.float32)
    gb2 = np.ascontiguousarray(np.stack([g2, be2], 1), dtype=np.float32)

    in_maps = []
    for c in range(NCORES):
        b, h = divmod(c, 2)
        sl = slice(h * NH, (h + 1) * NH)
        x1 = xyz1[b][:, sl]
        x1aug = np.concatenate(
            [x1, np.ones((1, NH), np.float32), (x1 * x1).sum(0, keepdims=True)], 0
        )
        x2 = xyz2[b]
        x2aug = np.concatenate(
            [2.0 * x2, -(x2 * x2).sum(0, keepdims=True), -np.ones((1, S), np.float32)], 0
        )
        in_maps.append({
            "x1aug": np.ascontiguousarray(x1aug, np.float32),
            "x2aug": np.ascontiguousarray(x2aug, np.float32),
            "p1": np.ascontiguousarray(points1[b][:, sl], np.float32),
            "p2t": np.ascontiguousarray(points2[b].T, np.float32),
            "rhs1": rhs1,
            "rhs2": rhs2,
            "g1b": g1bc,
            "be1b": be1bc,
            "gb2": gb2,
        })
    return in_maps


def _run(in_maps, trace=False, **kw):
    from concourse import bass_utils
    return bass_utils.run_bass_kernel_spmd(
        _get_nc(), in_maps, core_ids=list(range(NCORES)), trace=trace, **kw
    )


def kernel(**inputs):
    in_maps = _make_in_maps(inputs)
    res = _run(in_maps)
    out = np.empty((B, H2, N), np.float32)
    for c in range(NCORES):
        b, h = divmod(c, 2)
        out[b][:, h * NH:(h + 1) * NH] = np.asarray(res.results[c]["out"])
    return out
